# revision 2
# baseline (speedup 1.0000x reference)
"""Trainium2 Bass kernel for nn_CGPBlock (attention block with 1x1-conv QKV).

Key optimizations over the ACT-paced baseline:
  - dual-engine exp: ~1/6 of the 128 exps run on the DVE as one tensor_scalar
    op via the Schraudolph int-exp trick: int16(e*184.665 + 16250.5) bits are
    bf16(~e^e) with +-3% multiplicative error (harmless here: out = pose +
    gamma*va with |gamma|~0.1 and absmax tolerance 1e-1).
  - one flat software pipeline over all 128 j-tiles (no per-chunk pipeline
    drains); previous chunk's Z-flush/normalization pegged into the next
    chunk's stream.
  - fp8(e4m3) conv inputs halve the startup DMA bytes (conv error ~1e-3 in
    the final output, well within tolerance).
  - bv folded into the v-conv drain bias: out = (va + bv*Z)*(gamma/Z) + pose
    so normalization is one bf16 2x multiply + one add (no gamma/bfin ops).
  - normalization (incl. last chunk) uses the DRAM-roundtrip broadcast of
    gamma/Z in bf16; Z group matmuls deferred for cross-engine slack.
"""

import numpy as np
import ml_dtypes

import concourse.bacc as bacc
import concourse.tile as tile
from concourse import mybir
from concourse.bass_utils import run_bass_kernel_spmd

F32 = mybir.dt.float32
BF16 = mybir.dt.bfloat16
FP8 = mybir.dt.float8e4
I16 = mybir.dt.int16
AF = mybir.ActivationFunctionType
ALU = mybir.AluOpType

B, C, L = 8, 128, 4096
CHUNK = 1024                # i-chunk width
NCH = L // CHUNK            # 4 chunks
NJT = L // 128              # 32 j-tiles
GROUP = 8                   # j-tiles per Z matmul group
NGRP = NJT // GROUP         # 4 groups per chunk
SKEW = 8                    # software pipeline depth (PE runs ahead of exp)

# Schraudolph constants: int16(e*SCH_S + SCH_B) bits == bf16(~exp(e))
SCH_S = 128.0 * 1.4426950408889634        # 128*log2(e)
SCH_B = 16250.5                           # 127<<7 minus error-centering 5.5

# which j-tiles exp on DVE (else ACT)
DVE_EXP_MOD = 6             # T % MOD == MOD-1 -> DVE
# Z pre-sum groups on GPSIMD (disabled: SBUF-port contention with the DVE
# makes GPSIMD adds a net loss)
GPSIMD_GROUPS = set()
WARMUP_MM = 8

_CACHE = {}


def _build():
    nc = bacc.Bacc("TRN2", target_bir_lowering=False, debug=False, num_devices=B)

    pose_d = nc.dram_tensor("pose", [C, L], F32, kind="ExternalInput").ap()
    pose8_d = nc.dram_tensor("pose8", [C, L], FP8, kind="ExternalInput").ap()
    id8_d = nc.dram_tensor("id8", [C, L], FP8, kind="ExternalInput").ap()
    wt_d = nc.dram_tensor("wt", [C, 3 * C], BF16, kind="ExternalInput").ap()
    bq_d = nc.dram_tensor("bq", [C, 1], F32, kind="ExternalInput").ap()
    bk_d = nc.dram_tensor("bk", [C, 1], F32, kind="ExternalInput").ap()
    bv_d = nc.dram_tensor("bv", [C, 1], F32, kind="ExternalInput").ap()
    gam_d = nc.dram_tensor("gam", [C, 1], F32, kind="ExternalInput").ap()
    out_d = nc.dram_tensor("out", [C, L], F32, kind="ExternalOutput").ap()

    with tile.TileContext(nc) as tc:
        with tc.tile_pool(name="res", bufs=1) as res:
            wt_sb = res.tile([C, 3 * C], BF16)
            nc.sync.dma_start(wt_sb, wt_d)
            bq_sb = res.tile([C, 1], F32)
            bk_sb = res.tile([C, 1], F32)
            bv_sb = res.tile([C, 1], F32)
            gam_sb = res.tile([C, 1], F32)
            ones_sb = res.tile([C, 1], BF16)
            nc.vector.memset(ones_sb, 1.0)
            onesr_sb = res.tile([1, C], F32)
            nc.vector.memset(onesr_sb, 1.0)

            def chunk_tiles(prefix, dtype):
                return [res.tile([C, CHUNK], dtype, name=f"{prefix}{i}")
                        for i in range(NCH)]

            pose_t = chunk_tiles("pose", F32)
            pose8_h = [res.tile([C, L // 2], FP8, name=f"pose8h{i}")
                       for i in range(2)]
            id8_h = [res.tile([C, L // 2], FP8, name=f"id8h{i}")
                     for i in range(2)]
            q_t = chunk_tiles("q", BF16)
            k_t = chunk_tiles("k", BF16)
            v_t = chunk_tiles("v", BF16)
            vt_t = chunk_tiles("vt", BF16)   # [j (partition), jt*128 + c]

            # input DMAs: one full-tensor transfer per input (4KB contiguous
            # per partition -- chunked transfers have 1KB rows and crawl at
            # <80GB/s), split across the two HWDGE queues. Tiny biases lead
            # the scalar queue so the conv drains never wait on them.
            nc.scalar.dma_start(bq_sb, bq_d)
            nc.scalar.dma_start(bk_sb, bk_d)
            nc.scalar.dma_start(bv_sb, bv_d)
            nc.scalar.dma_start(gam_sb, gam_d)
            for i in range(2):
                hl = slice(i * L // 2, (i + 1) * L // 2)
                nc.sync.dma_start(id8_h[i], id8_d[:, hl])
                nc.sync.dma_start(pose8_h[i], pose8_d[:, hl])
            # pose fp32 (residual, 2MB) rides the gpsimd SWDGE queue but its
            # triggers are held back behind a dependency on the last conv so
            # it never competes with the critical input loads (all DMA queues
            # share one engine); it is only needed at normalization time


            # PE clock-gate pre-warm during the input DMAs
            warm_sb = res.tile([C, 512], BF16)
            nc.vector.memset(warm_sb, 0.0)
            with tc.tile_pool(name="warm_ps", bufs=1, space="PSUM") as warm_ps:
                wp = warm_ps.tile([1, 512], F32)
                for _ in range(WARMUP_MM):
                    nc.tensor.matmul(wp, lhsT=ones_sb, rhs=warm_sb,
                                     start=True, stop=True)

            wqT = wt_sb[:, 0:C]
            wkT = wt_sb[:, C:2 * C]
            wvT = wt_sb[:, 2 * C:3 * C]

            # ---- QKV convs (1x1 = channel-mixing matmuls) ----
            # drains are split 512/512 across ACT and DVE so the conv phase
            # is DMA-paced, not drain-paced. v' = v + bv folds the va bias:
            # va'*(gamma/Z) = gamma*va/Z + gamma*bv.
            H0, H1 = slice(0, 512), slice(512, 1024)

            def emit_conv1(p, w, rhs, dst, bias):
                for h in range(CHUNK // 512):
                    hs = slice(h * 512, (h + 1) * 512)
                    nc.tensor.matmul(p[:, hs], lhsT=w, rhs=rhs[:, hs],
                                     start=True, stop=True)
                nc.scalar.activation(dst[:, H0], p[:, H0], AF.Identity,
                                     bias=bias)
                nc.vector.tensor_scalar_add(dst[:, H1], p[:, H1], bias)

            def emit_conv(ch, mk):
                sl = slice((ch % 2) * CHUNK, (ch % 2 + 1) * CHUNK)
                emit_conv1(mk("kp"), wkT, id8_h[ch // 2][:, sl], k_t[ch],
                           bk_sb)
                emit_conv1(mk("vp"), wvT, pose8_h[ch // 2][:, sl], v_t[ch],
                           bv_sb)
                emit_conv1(mk("qp"), wqT, pose8_h[ch // 2][:, sl], q_t[ch],
                           bq_sb)
                if ch == NCH - 1:
                    # gate the pose fp32 loads behind the last conv: the tiny
                    # copies write into each pose tile, so the DMAs inherit a
                    # WAW dependency the scheduler cannot hoist past
                    for pch in range(NCH):
                        nc.gpsimd.tensor_copy(pose_t[pch][0:1, 0:2],
                                              q_t[ch][0:1, 0:2])
                    for pch in range(NCH):
                        psl = slice(pch * CHUNK, (pch + 1) * CHUNK)
                        nc.gpsimd.dma_start(pose_t[pch], pose_d[:, psl])
                # vT tiles: blockwise DMA xbar transpose
                nc.sync.dma_start_transpose(
                    vt_t[ch].rearrange("p (t c) -> p t c", c=C),
                    v_t[ch])

            with tc.tile_pool(name="conv_ps", bufs=3, space="PSUM") as conv_ps:
                emit_conv(0, lambda nm: conv_ps.tile([C, CHUNK], F32,
                                                     tag="cv", name=nm))

            # ---- attention: one flat software pipeline over all 128 j-tiles
            # so chunk boundaries never drain the PE/ACT pipelines ----
            TOT = NCH * NJT
            with (
                tc.tile_pool(name="et_ps", bufs=2, space="PSUM") as et_ps,
                tc.tile_pool(name="va_ps", bufs=1, space="PSUM") as va_ps,
                tc.tile_pool(name="z_ps", bufs=1, space="PSUM") as z_ps,
                tc.tile_pool(name="pt_sb", bufs=SKEW + GROUP + 3) as pt_pool,
                tc.tile_pool(name="qs_sb", bufs=2) as qs_pool,
                tc.tile_pool(name="nrm", bufs=2) as nrm,
                tc.tile_pool(name="outb", bufs=2) as outb,
                tc.tile_pool(name="dramp", bufs=2, space="DRAM") as dramp,
            ):
                va = {}         # ch -> PSUM accumulator
                zt = {}         # ch -> PSUM Z row
                va_sb = {}      # ch -> drained va in SBUF (bf16)
                rzbs = {}       # ch -> broadcast gamma/Z (bf16)
                pts = {}        # global tile idx -> pt
                s8s = {}        # (ch, g) -> pre-summed [C, CHUNK] bf16
                zmm_done = {}   # ch -> groups whose Z matmul was emitted
                z_open = {}     # ch -> z accumulation started

                def emit_zmm_g(ch, g, pe_stop):
                    zmm_done[ch] += 1
                    s8 = s8s.pop((ch, g))
                    # first emitted group must clear BOTH 512-halves (banks)
                    first = not z_open[ch]
                    z_open[ch] = True
                    for h in range(CHUNK // 512):
                        hs = slice(h * 512, (h + 1) * 512)
                        nc.tensor.matmul(zt[ch][0:1, hs], lhsT=ones_sb,
                                         rhs=s8[:, hs],
                                         start=first, stop=pe_stop)

                def flush_zmms(ch, for_tail=False):
                    pend = sorted(g for (c, g) in s8s if c == ch
                                  and not isinstance(g, str))
                    for i, g in enumerate(pend):
                        last_one = (i == len(pend) - 1) and not for_tail
                        emit_zmm_g(ch, g, last_one)

                def emit_rz_broadcast(ch, dma):
                    # 1/Z broadcast across partitions via DRAM roundtrip
                    # (gamma is folded into the va_sb drain)
                    rz = nrm.tile([1, CHUNK], F32, tag="rz")
                    nc.vector.reciprocal_approx_fast(rz, zt[ch])
                    zd = dramp.tile([1, CHUNK], F32)
                    dma.dma_start(zd, rz)
                    rzb = nrm.tile([C, CHUNK], F32, tag="rzb")
                    dma.dma_start(rzb, zd.to_broadcast([C, CHUNK]))
                    rzbs[ch] = rzb

                def emit_norm(ch, dma):
                    # out = va'*(gamma/Z) + pose; one full-width out DMA (4KB
                    # rows -- half-width writes halve the DMA packet size)
                    rzb = rzbs.pop(ch)
                    vs = va_sb.pop(ch)
                    isl = slice(ch * CHUNK, (ch + 1) * CHUNK)
                    t = nrm.tile([C, CHUNK], F32, tag="t")
                    o = outb.tile([C, CHUNK], F32)
                    nc.vector.tensor_mul(t, vs, rzb)
                    nc.vector.tensor_add(o, t, pose_t[ch])
                    dma.dma_start(out_d[:, isl], o)

                def skew_at(t):
                    return max(1, min(SKEW, TOT + 1 - t))

                lag_ptr = 0
                for T in range(TOT + 1):
                    ch, jt = divmod(T, NJT) if T < TOT else (NCH - 1, NJT)
                    if T < TOT:
                        if jt == 0:
                            va[ch] = va_ps.tile([C, CHUNK], F32, name="va")
                            zt[ch] = z_ps.tile([1, CHUNK], F32, name="z")
                            zmm_done[ch] = 0
                            z_open[ch] = False
                        ksl = slice((jt % GROUP) * 128, (jt % GROUP + 1) * 128)
                        et = et_ps.tile([C, CHUNK], F32)
                        for h in range(CHUNK // 512):
                            hs = slice(h * 512, (h + 1) * 512)
                            nc.tensor.matmul(
                                et[:, hs], lhsT=k_t[jt // GROUP][:, ksl],
                                rhs=q_t[ch][:, hs],
                                start=True, stop=True)
                        pt = pt_pool.tile([C, CHUNK], BF16)
                        if (T >= TOT - 6 and (TOT - 1 - T) % 2 == 0
                                ) or T % DVE_EXP_MOD == DVE_EXP_MOD - 1:
                            # Schraudolph int-exp on the DVE (also forced for
                            # the final tile so ACT and DVE finish together)
                            nc.vector.tensor_scalar(
                                pt.bitcast(I16), et, SCH_S, SCH_B,
                                op0=ALU.mult, op1=ALU.add)
                        else:
                            nc.scalar.activation(pt, et, AF.Exp)
                        pts[T] = pt

                        # convs for chunk m ride inside the pipeline (k_t[m]
                        # is first needed at T = 8m), borrowing et-pool slots
                        if jt in (2, 10, 18) and ch == 0:
                            m = jt // 8 + 1
                            emit_conv(m, lambda nm: et_ps.tile(
                                [C, CHUNK], F32, tag="et", name=nm))

                        # cross-chunk tail work for the previous chunk, pegged
                        # early in this chunk so no engine ever waits long
                        if jt == 12 and ch >= 1:
                            pch = ch - 1
                            flush_zmms(pch)
                            emit_rz_broadcast(pch, nc.sync)
                        if jt == 20 and ch >= 1:
                            emit_norm(ch - 1, dma=nc.sync)

                    while lag_ptr <= min(T - skew_at(T), TOT - 1):
                        lg = lag_ptr
                        lag_ptr += 1
                        lch, ljt = divmod(lg, NJT)
                        vsl = slice((ljt % GROUP) * 128,
                                    (ljt % GROUP + 1) * 128)
                        pt = pts[lg]
                        for h in range(CHUNK // 512):
                            hs = slice(h * 512, (h + 1) * 512)
                            nc.tensor.matmul(
                                va[lch][:, hs],
                                lhsT=vt_t[ljt // GROUP][:, vsl],
                                rhs=pt[:, hs],
                                start=(ljt == 0),
                                stop=(ljt == NJT - 1))
                        if ljt == NJT - 1:
                            # drain va to SBUF right away (frees the PSUM bank
                            # for the next chunk) -- before the group-3 adds
                            # so the PE's next-chunk va never waits on them
                            vs = nrm.tile([C, CHUNK], F32, tag="va_sb",
                                          name="va_sb")
                            nc.vector.tensor_scalar_mul(vs, va.pop(lch),
                                                        gam_sb)
                            va_sb[lch] = vs
                        g = ljt // GROUP
                        direct_z = lch == NCH - 1 and ljt >= NJT - GROUP
                        if direct_z:
                            # kernel tail: per-tile Z matmuls so Z closes
                            # right behind the last exp
                            pts.pop(lg)
                            if ljt == NJT - GROUP:
                                flush_zmms(lch, for_tail=True)
                            for h in range(CHUNK // 512):
                                hs = slice(h * 512, (h + 1) * 512)
                                nc.tensor.matmul(zt[lch][0:1, hs],
                                                 lhsT=ones_sb,
                                                 rhs=pt[:, hs],
                                                 start=False,
                                                 stop=(ljt == NJT - 1))
                        elif ljt % GROUP == GROUP - 1:
                            # pre-sum tree over the group's 8 pt tiles
                            p = [pts.pop(lg - 7 + i) for i in range(8)]
                            eng = (nc.gpsimd if g in GPSIMD_GROUPS
                                   and lch < NCH - 1 else nc.vector)
                            a = []
                            for i in range(4):
                                t = qs_pool.tile([C, CHUNK], BF16,
                                                 tag=f"a{i}")
                                eng.tensor_tensor(
                                    t, p[2 * i], p[2 * i + 1], ALU.add)
                                a.append(t)
                            b0 = qs_pool.tile([C, CHUNK], BF16, tag="b0")
                            eng.tensor_tensor(b0, a[0], a[1], ALU.add)
                            b1 = qs_pool.tile([C, CHUNK], BF16, tag="b1")
                            eng.tensor_tensor(b1, a[2], a[3], ALU.add)
                            s8 = qs_pool.tile([C, CHUNK], BF16, tag="s8")
                            eng.tensor_tensor(s8, b0, b1, ALU.add)
                            s8s[lch, g] = s8
                            # defer DVE groups' Z matmuls one group for
                            # cross-engine slack (rest flush next chunk);
                            # GPSIMD groups only ever flush next chunk
                            if g in (1, 2) and not (
                                    lch == NCH - 1 and g == 2):
                                cand = min(gg for (c, gg) in s8s if c == lch
                                           and not isinstance(gg, str))
                                if cand not in GPSIMD_GROUPS:
                                    emit_zmm_g(lch, cand, False)

                # last chunk: broadcast gamma/Z on the (now idle) PE and
                # drain in 512-wide strips, out DMA on the idle scalar queue
                lch = NCH - 1
                rz = nrm.tile([1, CHUNK], F32, tag="rz")
                rzp = et_ps.tile([C, CHUNK], F32, tag="et", name="rzb_ps")
                t = nrm.tile([C, CHUNK], F32, tag="t")
                o = outb.tile([C, CHUNK], F32)
                vs = va_sb.pop(lch)
                halves = [slice(h * 512, (h + 1) * 512)
                          for h in range(CHUNK // 512)]
                for hs in halves:
                    nc.vector.reciprocal_approx_fast(
                        rz[0:1, hs], zt[lch][0:1, hs])
                for hs in halves:
                    nc.tensor.matmul(rzp[:, hs], lhsT=onesr_sb,
                                     rhs=rz[0:1, hs],
                                     start=True, stop=True)
                for h, hs in enumerate(halves):
                    ihs = slice(lch * CHUNK + hs.start, lch * CHUNK + hs.stop)
                    nc.vector.tensor_mul(t[:, hs], vs[:, hs], rzp[:, hs])
                    nc.vector.tensor_add(o[:, hs], t[:, hs],
                                         pose_t[lch][:, hs])
                    (nc.scalar if h == 0 else nc.sync).dma_start(
                        out_d[:, ihs], o[:, hs])

    nc.compile()
    return nc


def _get_nc():
    if "nc" not in _CACHE:
        _CACHE["nc"] = _build()
    return _CACHE["nc"]


def kernel(pose_f, id_f, Wq, bq, Wk, bk, Wv, bv, gamma, **run_kwargs):
    pose_f = np.asarray(pose_f, dtype=np.float32)
    id_f = np.asarray(id_f, dtype=np.float32)
    Wq = np.asarray(Wq, dtype=np.float32)
    Wk = np.asarray(Wk, dtype=np.float32)
    Wv = np.asarray(Wv, dtype=np.float32)
    bq = np.asarray(bq, dtype=np.float32)
    bk = np.asarray(bk, dtype=np.float32)
    bv = np.asarray(bv, dtype=np.float32)
    g = float(np.asarray(gamma, dtype=np.float32).reshape(-1)[0])

    bf = ml_dtypes.bfloat16
    f8 = ml_dtypes.float8_e4m3fn
    wt = np.concatenate([Wq.T, Wk.T, Wv.T], axis=1).astype(bf)  # [C_in, 3C]
    pose8 = pose_f.astype(f8)
    id8 = id_f.astype(f8)
    bq_c = np.ascontiguousarray(bq.reshape(C, 1))
    bk_c = np.ascontiguousarray(bk.reshape(C, 1))
    bv_c = np.ascontiguousarray(bv.reshape(C, 1))
    gam = np.full((C, 1), g, dtype=np.float32)

    in_maps = []
    for b in range(B):
        in_maps.append({
            "pose": pose_f[b],
            "pose8": pose8[b],
            "id8": id8[b],
            "wt": wt,
            "bq": bq_c,
            "bk": bk_c,
            "bv": bv_c,
            "gam": gam,
        })

    nc = _get_nc()
    res = run_bass_kernel_spmd(nc, in_maps, core_ids=list(range(B)), **run_kwargs)
    out = np.stack([res.results[b]["out"] for b in range(B)], axis=0)
    if run_kwargs:
        _CACHE["last_result"] = res
    return out


# revision 3
# speedup vs baseline: 1.0131x; 1.0131x over previous
"""Trainium2 Bass kernel for nn_CGPBlock (attention block with 1x1-conv QKV).

Key optimizations over the ACT-paced baseline:
  - dual-engine exp: ~1/6 of the 128 exps run on the DVE as one tensor_scalar
    op via the Schraudolph int-exp trick: int16(e*184.665 + 16250.5) bits are
    bf16(~e^e) with +-3% multiplicative error (harmless here: out = pose +
    gamma*va with |gamma|~0.1 and absmax tolerance 1e-1).
  - one flat software pipeline over all 128 j-tiles (no per-chunk pipeline
    drains); previous chunk's Z-flush/normalization pegged into the next
    chunk's stream.
  - fp8(e4m3) conv inputs halve the startup DMA bytes (conv error ~1e-3 in
    the final output, well within tolerance).
  - bv folded into the v-conv drain bias: out = (va + bv*Z)*(gamma/Z) + pose
    so normalization is one bf16 2x multiply + one add (no gamma/bfin ops).
  - normalization (incl. last chunk) uses the DRAM-roundtrip broadcast of
    gamma/Z in bf16; Z group matmuls deferred for cross-engine slack.
"""

import numpy as np
import ml_dtypes

import concourse.bacc as bacc
import concourse.tile as tile
from concourse import mybir
from concourse.bass_utils import run_bass_kernel_spmd

F32 = mybir.dt.float32
BF16 = mybir.dt.bfloat16
FP8 = mybir.dt.float8e4
I16 = mybir.dt.int16
AF = mybir.ActivationFunctionType
ALU = mybir.AluOpType

B, C, L = 8, 128, 4096
CHUNK = 1024                # i-chunk width
NCH = L // CHUNK            # 4 chunks
NJT = L // 128              # 32 j-tiles
GROUP = 8                   # j-tiles per Z matmul group
NGRP = NJT // GROUP         # 4 groups per chunk
SKEW = 8                    # software pipeline depth (PE runs ahead of exp)

# Schraudolph constants: int16(e*SCH_S + SCH_B) bits == bf16(~exp(e))
SCH_S = 128.0 * 1.4426950408889634        # 128*log2(e)
SCH_B = 16250.5                           # 127<<7 minus error-centering 5.5

# which j-tiles exp on DVE (else ACT)
DVE_EXP_MOD = 6             # T % MOD == MOD-1 -> DVE
# Z pre-sum groups on GPSIMD (disabled: SBUF-port contention with the DVE
# makes GPSIMD adds a net loss)
GPSIMD_GROUPS = set()
WARMUP_MM = 8

_CACHE = {}


def _build():
    nc = bacc.Bacc("TRN2", target_bir_lowering=False, debug=False, num_devices=B)

    pose_d = nc.dram_tensor("pose", [C, L], F32, kind="ExternalInput").ap()
    pose8_d = nc.dram_tensor("pose8", [C, L], FP8, kind="ExternalInput").ap()
    id8_d = nc.dram_tensor("id8", [C, L], FP8, kind="ExternalInput").ap()
    wt_d = nc.dram_tensor("wt", [C, 3 * C], BF16, kind="ExternalInput").ap()
    bq_d = nc.dram_tensor("bq", [C, 1], F32, kind="ExternalInput").ap()
    bk_d = nc.dram_tensor("bk", [C, 1], F32, kind="ExternalInput").ap()
    bv_d = nc.dram_tensor("bv", [C, 1], F32, kind="ExternalInput").ap()
    gam_d = nc.dram_tensor("gam", [C, 1], F32, kind="ExternalInput").ap()
    out_d = nc.dram_tensor("out", [C, L], F32, kind="ExternalOutput").ap()

    with tile.TileContext(nc) as tc:
        with tc.tile_pool(name="res", bufs=1) as res:
            wt_sb = res.tile([C, 3 * C], BF16)
            nc.gpsimd.dma_start(wt_sb, wt_d)
            bq_sb = res.tile([C, 1], F32)
            bk_sb = res.tile([C, 1], F32)
            bv_sb = res.tile([C, 1], F32)
            gam_sb = res.tile([C, 1], F32)
            ones_sb = res.tile([C, 1], BF16)
            nc.vector.memset(ones_sb, 1.0)
            onesr_sb = res.tile([1, C], F32)
            nc.vector.memset(onesr_sb, 1.0)

            def chunk_tiles(prefix, dtype):
                return [res.tile([C, CHUNK], dtype, name=f"{prefix}{i}")
                        for i in range(NCH)]

            pose_t = chunk_tiles("pose", F32)
            pose8_h = [res.tile([C, L // 2], FP8, name=f"pose8h{i}")
                       for i in range(2)]
            id8_h = [res.tile([C, L // 2], FP8, name=f"id8h{i}")
                     for i in range(2)]
            q_t = chunk_tiles("q", BF16)
            k_t = chunk_tiles("k", BF16)
            v_t = chunk_tiles("v", BF16)
            vt_t = chunk_tiles("vt", BF16)   # [j (partition), jt*128 + c]

            # input DMAs: one full-tensor transfer per input (4KB contiguous
            # per partition -- chunked transfers have 1KB rows and crawl at
            # <80GB/s), split across the two HWDGE queues. Tiny biases lead
            # the scalar queue so the conv drains never wait on them.
            nc.scalar.dma_start(bq_sb, bq_d)
            nc.scalar.dma_start(bk_sb, bk_d)
            nc.scalar.dma_start(bv_sb, bv_d)
            nc.scalar.dma_start(gam_sb, gam_d)
            for i in range(2):
                hl = slice(i * L // 2, (i + 1) * L // 2)
                nc.sync.dma_start(id8_h[i], id8_d[:, hl])
                nc.sync.dma_start(pose8_h[i], pose8_d[:, hl])
            # pose fp32 (residual, 2MB) rides the gpsimd SWDGE queue but its
            # triggers are held back behind a dependency on the last conv so
            # it never competes with the critical input loads (all DMA queues
            # share one engine); it is only needed at normalization time


            # PE clock-gate pre-warm during the input DMAs
            warm_sb = res.tile([C, 512], BF16)
            nc.vector.memset(warm_sb, 0.0)
            with tc.tile_pool(name="warm_ps", bufs=1, space="PSUM") as warm_ps:
                wp = warm_ps.tile([1, 512], F32)
                for _ in range(WARMUP_MM):
                    nc.tensor.matmul(wp, lhsT=ones_sb, rhs=warm_sb,
                                     start=True, stop=True)

            wqT = wt_sb[:, 0:C]
            wkT = wt_sb[:, C:2 * C]
            wvT = wt_sb[:, 2 * C:3 * C]

            # ---- QKV convs (1x1 = channel-mixing matmuls) ----
            # drains are split 512/512 across ACT and DVE so the conv phase
            # is DMA-paced, not drain-paced. v' = v + bv folds the va bias:
            # va'*(gamma/Z) = gamma*va/Z + gamma*bv.
            H0, H1 = slice(0, 512), slice(512, 1024)

            def emit_conv1(p, w, rhs, dst, bias):
                for h in range(CHUNK // 512):
                    hs = slice(h * 512, (h + 1) * 512)
                    nc.tensor.matmul(p[:, hs], lhsT=w, rhs=rhs[:, hs],
                                     start=True, stop=True)
                nc.scalar.activation(dst[:, H0], p[:, H0], AF.Identity,
                                     bias=bias)
                nc.vector.tensor_scalar_add(dst[:, H1], p[:, H1], bias)

            def emit_conv(ch, mk):
                sl = slice((ch % 2) * CHUNK, (ch % 2 + 1) * CHUNK)
                emit_conv1(mk("kp"), wkT, id8_h[ch // 2][:, sl], k_t[ch],
                           bk_sb)
                emit_conv1(mk("vp"), wvT, pose8_h[ch // 2][:, sl], v_t[ch],
                           bv_sb)
                emit_conv1(mk("qp"), wqT, pose8_h[ch // 2][:, sl], q_t[ch],
                           bq_sb)
                if ch == NCH - 1:
                    # gate the pose fp32 loads behind the last conv: the tiny
                    # copies write into each pose tile, so the DMAs inherit a
                    # WAW dependency the scheduler cannot hoist past
                    for pch in range(NCH):
                        nc.gpsimd.tensor_copy(pose_t[pch][0:1, 0:2],
                                              q_t[ch][0:1, 0:2])
                    for pch in range(NCH):
                        psl = slice(pch * CHUNK, (pch + 1) * CHUNK)
                        nc.gpsimd.dma_start(pose_t[pch], pose_d[:, psl])
                # vT tiles: blockwise DMA xbar transpose
                nc.sync.dma_start_transpose(
                    vt_t[ch].rearrange("p (t c) -> p t c", c=C),
                    v_t[ch])

            with tc.tile_pool(name="conv_ps", bufs=3, space="PSUM") as conv_ps:
                emit_conv(0, lambda nm: conv_ps.tile([C, CHUNK], F32,
                                                     tag="cv", name=nm))

            # ---- attention: one flat software pipeline over all 128 j-tiles
            # so chunk boundaries never drain the PE/ACT pipelines ----
            TOT = NCH * NJT
            with (
                tc.tile_pool(name="et_ps", bufs=2, space="PSUM") as et_ps,
                tc.tile_pool(name="va_ps", bufs=1, space="PSUM") as va_ps,
                tc.tile_pool(name="z_ps", bufs=1, space="PSUM") as z_ps,
                tc.tile_pool(name="pt_sb", bufs=SKEW + GROUP + 3) as pt_pool,
                tc.tile_pool(name="qs_sb", bufs=2) as qs_pool,
                tc.tile_pool(name="nrm", bufs=2) as nrm,
                tc.tile_pool(name="outb", bufs=2) as outb,
                tc.tile_pool(name="dramp", bufs=2, space="DRAM") as dramp,
            ):
                va = {}         # ch -> PSUM accumulator
                zt = {}         # ch -> PSUM Z row
                va_sb = {}      # ch -> drained va in SBUF (bf16)
                rzbs = {}       # ch -> broadcast gamma/Z (bf16)
                pts = {}        # global tile idx -> pt
                s8s = {}        # (ch, g) -> pre-summed [C, CHUNK] bf16
                zmm_done = {}   # ch -> groups whose Z matmul was emitted
                z_open = {}     # ch -> z accumulation started

                def emit_zmm_g(ch, g, pe_stop):
                    zmm_done[ch] += 1
                    s8 = s8s.pop((ch, g))
                    # first emitted group must clear BOTH 512-halves (banks)
                    first = not z_open[ch]
                    z_open[ch] = True
                    for h in range(CHUNK // 512):
                        hs = slice(h * 512, (h + 1) * 512)
                        nc.tensor.matmul(zt[ch][0:1, hs], lhsT=ones_sb,
                                         rhs=s8[:, hs],
                                         start=first, stop=pe_stop)

                def flush_zmms(ch, for_tail=False):
                    pend = sorted(g for (c, g) in s8s if c == ch
                                  and not isinstance(g, str))
                    for i, g in enumerate(pend):
                        last_one = (i == len(pend) - 1) and not for_tail
                        emit_zmm_g(ch, g, last_one)

                def emit_rz_broadcast(ch, dma):
                    # 1/Z broadcast across partitions via DRAM roundtrip
                    # (gamma is folded into the va_sb drain)
                    rz = nrm.tile([1, CHUNK], F32, tag="rz")
                    nc.vector.reciprocal_approx_fast(rz, zt[ch])
                    zd = dramp.tile([1, CHUNK], F32)
                    dma.dma_start(zd, rz)
                    rzb = nrm.tile([C, CHUNK], F32, tag="rzb")
                    dma.dma_start(rzb, zd.to_broadcast([C, CHUNK]))
                    rzbs[ch] = rzb

                def emit_norm(ch, dma):
                    # out = va'*(gamma/Z) + pose; one full-width out DMA (4KB
                    # rows -- half-width writes halve the DMA packet size)
                    rzb = rzbs.pop(ch)
                    vs = va_sb.pop(ch)
                    isl = slice(ch * CHUNK, (ch + 1) * CHUNK)
                    t = nrm.tile([C, CHUNK], F32, tag="t")
                    o = outb.tile([C, CHUNK], F32)
                    nc.vector.tensor_mul(t, vs, rzb)
                    nc.vector.tensor_add(o, t, pose_t[ch])
                    dma.dma_start(out_d[:, isl], o)

                def skew_at(t):
                    return max(1, min(SKEW, TOT + 1 - t))

                lag_ptr = 0
                for T in range(TOT + 1):
                    ch, jt = divmod(T, NJT) if T < TOT else (NCH - 1, NJT)
                    if T < TOT:
                        if jt == 0:
                            va[ch] = va_ps.tile([C, CHUNK], F32, name="va")
                            zt[ch] = z_ps.tile([1, CHUNK], F32, name="z")
                            zmm_done[ch] = 0
                            z_open[ch] = False
                        ksl = slice((jt % GROUP) * 128, (jt % GROUP + 1) * 128)
                        et = et_ps.tile([C, CHUNK], F32)
                        for h in range(CHUNK // 512):
                            hs = slice(h * 512, (h + 1) * 512)
                            nc.tensor.matmul(
                                et[:, hs], lhsT=k_t[jt // GROUP][:, ksl],
                                rhs=q_t[ch][:, hs],
                                start=True, stop=True)
                        pt = pt_pool.tile([C, CHUNK], BF16)
                        if (T >= TOT - 6 and (TOT - 1 - T) % 2 == 0
                                ) or T % DVE_EXP_MOD == DVE_EXP_MOD - 1:
                            # Schraudolph int-exp on the DVE (also forced for
                            # the final tile so ACT and DVE finish together)
                            nc.vector.tensor_scalar(
                                pt.bitcast(I16), et, SCH_S, SCH_B,
                                op0=ALU.mult, op1=ALU.add)
                        else:
                            nc.scalar.activation(pt, et, AF.Exp)
                        pts[T] = pt

                        # convs for chunk m ride inside the pipeline (k_t[m]
                        # is first needed at T = 8m), borrowing et-pool slots
                        if jt in (2, 10, 18) and ch == 0:
                            m = jt // 8 + 1
                            emit_conv(m, lambda nm: et_ps.tile(
                                [C, CHUNK], F32, tag="et", name=nm))

                        # cross-chunk tail work for the previous chunk, pegged
                        # early in this chunk so no engine ever waits long
                        if jt == 12 and ch >= 1:
                            pch = ch - 1
                            flush_zmms(pch)
                            emit_rz_broadcast(pch, nc.sync)
                        if jt == 20 and ch >= 1:
                            emit_norm(ch - 1, dma=nc.sync)

                    while lag_ptr <= min(T - skew_at(T), TOT - 1):
                        lg = lag_ptr
                        lag_ptr += 1
                        lch, ljt = divmod(lg, NJT)
                        vsl = slice((ljt % GROUP) * 128,
                                    (ljt % GROUP + 1) * 128)
                        pt = pts[lg]
                        for h in range(CHUNK // 512):
                            hs = slice(h * 512, (h + 1) * 512)
                            nc.tensor.matmul(
                                va[lch][:, hs],
                                lhsT=vt_t[ljt // GROUP][:, vsl],
                                rhs=pt[:, hs],
                                start=(ljt == 0),
                                stop=(ljt == NJT - 1))
                        if ljt == NJT - 1:
                            # drain va to SBUF right away (frees the PSUM bank
                            # for the next chunk) -- before the group-3 adds
                            # so the PE's next-chunk va never waits on them
                            vs = nrm.tile([C, CHUNK], F32, tag="va_sb",
                                          name="va_sb")
                            nc.vector.tensor_scalar_mul(vs, va.pop(lch),
                                                        gam_sb)
                            va_sb[lch] = vs
                        g = ljt // GROUP
                        direct_z = lch == NCH - 1 and ljt >= NJT - GROUP
                        if direct_z:
                            # kernel tail: per-tile Z matmuls so Z closes
                            # right behind the last exp
                            pts.pop(lg)
                            if ljt == NJT - GROUP:
                                flush_zmms(lch, for_tail=True)
                            for h in range(CHUNK // 512):
                                hs = slice(h * 512, (h + 1) * 512)
                                nc.tensor.matmul(zt[lch][0:1, hs],
                                                 lhsT=ones_sb,
                                                 rhs=pt[:, hs],
                                                 start=False,
                                                 stop=(ljt == NJT - 1))
                        elif ljt % GROUP == GROUP - 1:
                            # pre-sum tree over the group's 8 pt tiles
                            p = [pts.pop(lg - 7 + i) for i in range(8)]
                            eng = (nc.gpsimd if g in GPSIMD_GROUPS
                                   and lch < NCH - 1 else nc.vector)
                            a = []
                            for i in range(4):
                                t = qs_pool.tile([C, CHUNK], BF16,
                                                 tag=f"a{i}")
                                eng.tensor_tensor(
                                    t, p[2 * i], p[2 * i + 1], ALU.add)
                                a.append(t)
                            b0 = qs_pool.tile([C, CHUNK], BF16, tag="b0")
                            eng.tensor_tensor(b0, a[0], a[1], ALU.add)
                            b1 = qs_pool.tile([C, CHUNK], BF16, tag="b1")
                            eng.tensor_tensor(b1, a[2], a[3], ALU.add)
                            s8 = qs_pool.tile([C, CHUNK], BF16, tag="s8")
                            eng.tensor_tensor(s8, b0, b1, ALU.add)
                            s8s[lch, g] = s8
                            if lch < NCH - 1:
                                # pair 8-groups into 16-wide Z groups: one
                                # extra DVE add halves the PE Z-matmul count;
                                # both Z groups flush next chunk
                                if g % 2 == 1:
                                    s16 = qs_pool.tile([C, CHUNK], BF16,
                                                       tag="s16")
                                    nc.vector.tensor_tensor(
                                        s16, s8s.pop((lch, g - 1)),
                                        s8s.pop((lch, g)), ALU.add)
                                    s8s[lch, g] = s16
                            elif g == 1:
                                # last chunk: emit group 0 promptly so the
                                # direct-z tail only waits on groups 1-2
                                emit_zmm_g(lch, 0, False)

                # last chunk: broadcast gamma/Z on the (now idle) PE and
                # drain in 512-wide strips, out DMA on the idle scalar queue
                lch = NCH - 1
                rz = nrm.tile([1, CHUNK], F32, tag="rz")
                rzp = et_ps.tile([C, CHUNK], F32, tag="et", name="rzb_ps")
                t = nrm.tile([C, CHUNK], F32, tag="t")
                o = outb.tile([C, CHUNK], F32)
                vs = va_sb.pop(lch)
                halves = [slice(h * 512, (h + 1) * 512)
                          for h in range(CHUNK // 512)]
                for hs in halves:
                    nc.vector.reciprocal_approx_fast(
                        rz[0:1, hs], zt[lch][0:1, hs])
                for hs in halves:
                    nc.tensor.matmul(rzp[:, hs], lhsT=onesr_sb,
                                     rhs=rz[0:1, hs],
                                     start=True, stop=True)
                for h, hs in enumerate(halves):
                    ihs = slice(lch * CHUNK + hs.start, lch * CHUNK + hs.stop)
                    nc.vector.tensor_mul(t[:, hs], vs[:, hs], rzp[:, hs])
                    nc.vector.tensor_add(o[:, hs], t[:, hs],
                                         pose_t[lch][:, hs])
                    (nc.scalar if h == 0 else nc.sync).dma_start(
                        out_d[:, ihs], o[:, hs])

    nc.compile()
    return nc


def _get_nc():
    if "nc" not in _CACHE:
        _CACHE["nc"] = _build()
    return _CACHE["nc"]


def kernel(pose_f, id_f, Wq, bq, Wk, bk, Wv, bv, gamma, **run_kwargs):
    pose_f = np.asarray(pose_f, dtype=np.float32)
    id_f = np.asarray(id_f, dtype=np.float32)
    Wq = np.asarray(Wq, dtype=np.float32)
    Wk = np.asarray(Wk, dtype=np.float32)
    Wv = np.asarray(Wv, dtype=np.float32)
    bq = np.asarray(bq, dtype=np.float32)
    bk = np.asarray(bk, dtype=np.float32)
    bv = np.asarray(bv, dtype=np.float32)
    g = float(np.asarray(gamma, dtype=np.float32).reshape(-1)[0])

    bf = ml_dtypes.bfloat16
    f8 = ml_dtypes.float8_e4m3fn
    wt = np.concatenate([Wq.T, Wk.T, Wv.T], axis=1).astype(bf)  # [C_in, 3C]
    pose8 = pose_f.astype(f8)
    id8 = id_f.astype(f8)
    bq_c = np.ascontiguousarray(bq.reshape(C, 1))
    bk_c = np.ascontiguousarray(bk.reshape(C, 1))
    bv_c = np.ascontiguousarray(bv.reshape(C, 1))
    gam = np.full((C, 1), g, dtype=np.float32)

    in_maps = []
    for b in range(B):
        in_maps.append({
            "pose": pose_f[b],
            "pose8": pose8[b],
            "id8": id8[b],
            "wt": wt,
            "bq": bq_c,
            "bk": bk_c,
            "bv": bv_c,
            "gam": gam,
        })

    nc = _get_nc()
    res = run_bass_kernel_spmd(nc, in_maps, core_ids=list(range(B)), **run_kwargs)
    out = np.stack([res.results[b]["out"] for b in range(B)], axis=0)
    if run_kwargs:
        _CACHE["last_result"] = res
    return out
